# revision 7
# baseline (speedup 1.0000x reference)
# Trainium2 Bass kernel for: embedding -> LSTM (last hidden) -> dense -> softmax
#
#   tokens [512, 512] int  -> emb lookup [B, T, 32] -> LSTM(64) last hidden
#   -> dense(3) -> softmax  => out [512, 3] f32
#
# Sharding: data-parallel over batch across 8 cores (64 rows each); embedding
# table + weights replicated.
#
# Per-core device program (fully unrolled over T=512 steps):
#   - gather emb rows for 2 timesteps at a time via indirect DMA -> [128, 32],
#     PE-transpose to x^T [32, 128], DMA the per-step [32, 64] slice into rows
#     64:96 of the step's rhs tile
#   - rhs tile [97, 64] = [h_t ; x_t^T ; 1]; one K=97 matmul per gate against
#     wcat = vstack(Wr, Wk, b) column slices -> z_i z_f in one psum tile,
#     z_g z_o in another (gates along the free dim so every elementwise op
#     stays on partitions 0:64)
#   - ACT: sigmoid(i|f) in one op, tanh(g), sigmoid(o); DVE: c' = f*c + i*g~,
#     h' = o * tanh(c') written straight into the next rhs tile
#   - dense head: one K=97 matmul with wdb = vstack(Wd, 0, bd) (the x rows
#     multiply zeros, the ones row adds bd), then softmax on device.

import numpy as np

VOCAB, EMB, HID, NCLS, B, T = 50000, 32, 64, 3, 512, 512
NCORES = 8
BL = B // NCORES  # 64 batch rows per core
KC = HID + EMB + 1  # 97: h rows, x rows, ones row
NH = 4  # h/rhs tile ring depth

_CACHE = {}


def build_program(t_steps=T):
    from contextlib import ExitStack

    import concourse.bass as bass
    import concourse.mybir as mybir
    import concourse.tile as tile
    from concourse import bacc
    from concourse.bass import ts
    from concourse.masks import make_identity

    f32 = mybir.dt.float32
    i32 = mybir.dt.int32
    npairs = t_steps // 2

    nc = bacc.Bacc("TRN2", target_bir_lowering=False, debug=False,
                   num_devices=NCORES)

    tok2_p = nc.declare_dram_parameter("tok2", [2 * BL, npairs], i32,
                                       isOutput=False)
    emb_p = nc.declare_dram_parameter("emb", [VOCAB, EMB], f32, isOutput=False)
    wcat_p = nc.declare_dram_parameter("wcat", [KC, 4 * HID], f32,
                                       isOutput=False)
    wdb_p = nc.declare_dram_parameter("wdb", [KC, NCLS], f32, isOutput=False)
    out_p = nc.declare_dram_parameter("out", [BL, NCLS], f32, isOutput=True)

    with ExitStack() as ctx:
        tc = ctx.enter_context(tile.TileContext(nc))
        consts = ctx.enter_context(tc.tile_pool(name="consts", bufs=1))
        state = ctx.enter_context(tc.tile_pool(name="state", bufs=1))
        gath_pool = ctx.enter_context(tc.tile_pool(name="gath", bufs=8))
        g_pool = ctx.enter_context(tc.tile_pool(name="gates", bufs=3))
        tmp_pool = ctx.enter_context(tc.tile_pool(name="tmps", bufs=3))
        pz_pool = ctx.enter_context(tc.tile_pool(name="pz", bufs=2,
                                                 space="PSUM"))
        pxt_pool = ctx.enter_context(tc.tile_pool(name="pxt", bufs=3,
                                                  space="PSUM"))
        head_pool = ctx.enter_context(tc.tile_pool(name="head", bufs=1))
        phead_pool = ctx.enter_context(tc.tile_pool(name="phead", bufs=1,
                                                    space="PSUM"))

        # ---- constants / weights in SBUF ----
        tok_sb = consts.tile([2 * BL, npairs], i32, name="tok_sb")
        nc.sync.dma_start(tok_sb[:], tok2_p[:])
        wcat_sb = consts.tile([KC, 4 * HID], f32, name="wcat_sb")
        nc.sync.dma_start(wcat_sb[:], wcat_p[:])
        wdb_sb = consts.tile([KC, NCLS], f32, name="wdb_sb")
        nc.sync.dma_start(wdb_sb[:], wdb_p[:])
        ident = consts.tile([128, 128], f32, name="ident")
        make_identity(nc, ident[:])

        # ---- persistent state ----
        # rhs ring: [h ; x^T ; 1] tiles; c ping-pong.
        hb = [state.tile([KC, BL], f32, name=f"hb{k}") for k in range(NH)]
        c_st = [state.tile([HID, BL], f32, name=f"c{k}") for k in (0, 1)]
        nc.vector.memset(hb[0][0:HID, :], 0.0)
        for k in range(NH):
            nc.vector.memset(hb[k][HID + EMB:KC, :], 1.0)
        nc.vector.memset(c_st[0][:], 0.0)

        pxt = None
        for t in range(t_steps):
            j, r = divmod(t, 2)
            if r == 0:
                # gather emb rows for steps (2j, 2j+1): row p of gath is
                # emb[tokens[p % 64, 2j + p // 64]]
                gath = gath_pool.tile([2 * BL, EMB], f32, name="gath")
                nc.gpsimd.indirect_dma_start(
                    out=gath[:],
                    out_offset=None,
                    in_=emb_p[:],
                    in_offset=bass.IndirectOffsetOnAxis(
                        ap=tok_sb[:, j:j + 1], axis=0),
                )
                # transpose -> [EMB, 128]: cols 0:64 = x_{2j}^T, rest x_{2j+1}^T
                pxt = pxt_pool.tile([EMB, 2 * BL], f32, name="pxt",
                                    space="PSUM")
                nc.tensor.matmul(pxt[:], lhsT=gath[:], rhs=ident[:],
                                 is_transpose=True, start=True, stop=True)
            # x_t^T into rows 64:96 of this step's rhs tile (partition-shifted
            # copy)
            nc.vector.tensor_copy(hb[t % NH][HID:HID + EMB, :],
                                  pxt[:, ts(r, BL)])

            h_in = hb[t % NH]
            h_out = hb[(t + 1) % NH]
            c_in = c_st[t % 2]
            c_out = c_st[(t + 1) % 2]

            # z_k = wcat[:, 64k:64k+64]^T @ [h; x; 1]   (bias via ones row)
            pzif = pz_pool.tile([HID, 2 * BL], f32, name="pzif", space="PSUM")
            pzgo = pz_pool.tile([HID, 2 * BL], f32, name="pzgo", space="PSUM")
            nc.tensor.matmul(pzif[:, 0:BL], lhsT=wcat_sb[:, 0:HID],
                             rhs=h_in[:], start=True, stop=True)
            nc.tensor.matmul(pzif[:, BL:2 * BL], lhsT=wcat_sb[:, HID:2 * HID],
                             rhs=h_in[:], start=True, stop=True)
            nc.tensor.matmul(pzgo[:, 0:BL], lhsT=wcat_sb[:, 2 * HID:3 * HID],
                             rhs=h_in[:], start=True, stop=True)
            nc.tensor.matmul(pzgo[:, BL:2 * BL], lhsT=wcat_sb[:, 3 * HID:4 * HID],
                             rhs=h_in[:], start=True, stop=True)

            # gates
            sif = g_pool.tile([HID, 2 * BL], f32, name="sif")
            nc.scalar.activation(sif[:], pzif[:],
                                 mybir.ActivationFunctionType.Sigmoid)
            tg = g_pool.tile([HID, BL], f32, name="tg")
            nc.scalar.activation(tg[:], pzgo[:, 0:BL],
                                 mybir.ActivationFunctionType.Tanh)
            so = g_pool.tile([HID, BL], f32, name="so")
            nc.scalar.activation(so[:], pzgo[:, BL:2 * BL],
                                 mybir.ActivationFunctionType.Sigmoid)

            # c' = f*c + i*g~ ; h' = o * tanh(c')
            v = tmp_pool.tile([HID, BL], f32, name="v")
            nc.vector.tensor_mul(v[:], sif[:, BL:2 * BL], c_in[:])
            u = tmp_pool.tile([HID, BL], f32, name="u")
            nc.vector.tensor_mul(u[:], sif[:, 0:BL], tg[:])
            nc.vector.tensor_add(c_out[:], u[:], v[:])
            thc = tmp_pool.tile([HID, BL], f32, name="thc")
            nc.scalar.activation(thc[:], c_out[:],
                                 mybir.ActivationFunctionType.Tanh)
            nc.vector.tensor_mul(h_out[0:HID, :], so[:], thc[:])

        # ---- dense head + softmax ----
        h_fin = hb[t_steps % NH]
        plog = phead_pool.tile([BL, NCLS], f32, name="plog", space="PSUM")
        nc.tensor.matmul(plog[:], lhsT=h_fin[:], rhs=wdb_sb[:], start=True,
                         stop=True)
        e = head_pool.tile([BL, NCLS], f32, name="e")
        nc.scalar.activation(e[:], plog[:], mybir.ActivationFunctionType.Exp)
        s = head_pool.tile([BL, 1], f32, name="s")
        nc.vector.tensor_reduce(s[:], e[:], axis=mybir.AxisListType.X,
                                op=mybir.AluOpType.add)
        rcp = head_pool.tile([BL, 1], f32, name="rcp")
        nc.vector.reciprocal(rcp[:], s[:])
        prob = head_pool.tile([BL, NCLS], f32, name="prob")
        nc.vector.tensor_scalar(prob[:], e[:], rcp[:, 0:1], None,
                                mybir.AluOpType.mult)
        nc.sync.dma_start(out_p[:], prob[:])

    nc.compile()
    return nc


def _host_prep(inputs, t_steps=T):
    tokens = np.ascontiguousarray(np.asarray(inputs["tokens"]).astype(np.int32))
    emb = np.ascontiguousarray(np.asarray(inputs["emb"], dtype=np.float32))
    Wk = np.asarray(inputs["Wk"], dtype=np.float32)
    Wr = np.asarray(inputs["Wr"], dtype=np.float32)
    b = np.asarray(inputs["b"], dtype=np.float32)
    Wd = np.asarray(inputs["Wd"], dtype=np.float32)
    bd = np.asarray(inputs["bd"], dtype=np.float32)

    # rhs rows: 0:64 h -> Wr, 64:96 x -> Wk, 96 ones -> b / bd
    wcat = np.ascontiguousarray(
        np.concatenate([Wr, Wk, b[None, :]], axis=0).astype(np.float32))
    wdb = np.ascontiguousarray(np.concatenate(
        [Wd, np.zeros((EMB, NCLS), np.float32), bd[None, :]],
        axis=0).astype(np.float32))

    in_maps = []
    for c in range(NCORES):
        shard = tokens[c * BL:(c + 1) * BL, :t_steps]  # [64, T]
        # tok2[r*64 + b, j] = shard[b, 2j + r]
        tok2 = np.ascontiguousarray(
            shard.reshape(BL, t_steps // 2, 2).transpose(2, 0, 1)
            .reshape(2 * BL, t_steps // 2))
        in_maps.append({"tok2": tok2, "emb": emb, "wcat": wcat, "wdb": wdb})
    return in_maps


def kernel(**inputs) -> np.ndarray:
    from concourse.bass_utils import run_bass_kernel_spmd

    if "prog" not in _CACHE:
        _CACHE["prog"] = build_program(T)
    nc = _CACHE["prog"]

    in_maps = _host_prep(inputs, T)
    res = run_bass_kernel_spmd(nc, in_maps, list(range(NCORES)))
    outs = [np.asarray(res.results[c]["out"]) for c in range(NCORES)]
    return np.concatenate(outs, axis=0).astype(np.float32)


# revision 9
# speedup vs baseline: 1.2639x; 1.2639x over previous
# Trainium2 Bass kernel for: embedding -> LSTM (last hidden) -> dense -> softmax
#
#   tokens [512, 512] int  -> emb lookup [B, T, 32] -> LSTM(64) last hidden
#   -> dense(3) -> softmax  => out [512, 3] f32
#
# Sharding: data-parallel over batch across 8 cores (64 rows each); embedding
# table + weights replicated.
#
# Per-core device program (fully unrolled over T=512 steps):
#   - gather emb rows for 2 timesteps at a time via indirect DMA -> [128, 32],
#     PE-transpose to x^T [32, 128], DMA the per-step [32, 64] slice into rows
#     64:96 of the step's rhs tile
#   - rhs tile [97, 64] = [h_t ; x_t^T ; 1]; one K=97 matmul per gate against
#     wcat = vstack(Wr, Wk, b) column slices -> z_i z_f in one psum tile,
#     z_g z_o in another (gates along the free dim so every elementwise op
#     stays on partitions 0:64)
#   - ACT: sigmoid(i|f) in one op, tanh(g), sigmoid(o); DVE: c' = f*c + i*g~,
#     h' = o * tanh(c') written straight into the next rhs tile
#   - dense head: one K=97 matmul with wdb = vstack(Wd, 0, bd) (the x rows
#     multiply zeros, the ones row adds bd), then softmax on device.

import numpy as np

VOCAB, EMB, HID, NCLS, B, T = 50000, 32, 64, 3, 512, 512
NCORES = 8
BL = B // NCORES  # 64 batch rows per core
KC = HID + EMB + 1  # 97: h rows, x rows, ones row
NH = 4  # h/rhs tile ring depth

_CACHE = {}


def build_program(t_steps=T):
    from contextlib import ExitStack

    import concourse.bass as bass
    import concourse.mybir as mybir
    import concourse.tile as tile
    from concourse import bacc
    from concourse.bass import ts
    from concourse.masks import make_identity

    f32 = mybir.dt.float32
    bf16 = mybir.dt.bfloat16
    i32 = mybir.dt.int32
    npairs = t_steps // 2

    nc = bacc.Bacc("TRN2", target_bir_lowering=False, debug=False,
                   num_devices=NCORES)

    tok2_p = nc.declare_dram_parameter("tok2", [2 * BL, npairs], i32,
                                       isOutput=False)
    emb_p = nc.declare_dram_parameter("emb", [VOCAB, EMB], bf16, isOutput=False)
    wcat_p = nc.declare_dram_parameter("wcat", [KC, 4 * HID], bf16,
                                       isOutput=False)
    wdb_p = nc.declare_dram_parameter("wdb", [KC, NCLS], f32, isOutput=False)
    out_p = nc.declare_dram_parameter("out", [BL, NCLS], f32, isOutput=True)

    with ExitStack() as ctx:
        tc = ctx.enter_context(tile.TileContext(nc))
        consts = ctx.enter_context(tc.tile_pool(name="consts", bufs=1))
        state = ctx.enter_context(tc.tile_pool(name="state", bufs=1))
        gath_pool = ctx.enter_context(tc.tile_pool(name="gath", bufs=8))
        g_pool = ctx.enter_context(tc.tile_pool(name="gates", bufs=3))
        tmp_pool = ctx.enter_context(tc.tile_pool(name="tmps", bufs=3))
        pz_pool = ctx.enter_context(tc.tile_pool(name="pz", bufs=2,
                                                 space="PSUM"))
        pxt_pool = ctx.enter_context(tc.tile_pool(name="pxt", bufs=3,
                                                  space="PSUM"))
        head_pool = ctx.enter_context(tc.tile_pool(name="head", bufs=1))
        phead_pool = ctx.enter_context(tc.tile_pool(name="phead", bufs=1,
                                                    space="PSUM"))

        # ---- constants / weights in SBUF ----
        tok_sb = consts.tile([2 * BL, npairs], i32, name="tok_sb")
        nc.sync.dma_start(tok_sb[:], tok2_p[:])
        wcat_sb = consts.tile([KC, 4 * HID], bf16, name="wcat_sb")
        nc.sync.dma_start(wcat_sb[:], wcat_p[:])
        wdb_sb = consts.tile([KC, NCLS], f32, name="wdb_sb")
        nc.sync.dma_start(wdb_sb[:], wdb_p[:])
        ident = consts.tile([128, 128], bf16, name="ident")
        make_identity(nc, ident[:])

        # ---- persistent state ----
        # rhs ring: [h ; x^T ; 1] tiles; c ping-pong.
        hb = [state.tile([KC, BL], bf16, name=f"hb{k}") for k in range(NH)]
        c_st = [state.tile([HID, BL], f32, name=f"c{k}") for k in (0, 1)]
        nc.vector.memset(hb[0][0:HID, :], 0.0)
        for k in range(NH):
            nc.vector.memset(hb[k][HID + EMB:KC, :], 1.0)
        nc.vector.memset(c_st[0][:], 0.0)

        pxt = None
        for t in range(t_steps):
            j, r = divmod(t, 2)
            if r == 0:
                # gather emb rows for steps (2j, 2j+1): row p of gath is
                # emb[tokens[p % 64, 2j + p // 64]]
                gath = gath_pool.tile([2 * BL, EMB], bf16, name="gath")
                nc.gpsimd.indirect_dma_start(
                    out=gath[:],
                    out_offset=None,
                    in_=emb_p[:],
                    in_offset=bass.IndirectOffsetOnAxis(
                        ap=tok_sb[:, j:j + 1], axis=0),
                )
                # transpose -> [EMB, 128]: cols 0:64 = x_{2j}^T, rest x_{2j+1}^T
                pxt = pxt_pool.tile([EMB, 2 * BL], bf16, name="pxt",
                                    space="PSUM")
                nc.tensor.matmul(pxt[:], lhsT=gath[:], rhs=ident[:],
                                 is_transpose=True, start=True, stop=True)
            # x_t^T into rows 64:96 of this step's rhs tile (partition-shifted
            # copy)
            nc.vector.tensor_copy(hb[t % NH][HID:HID + EMB, :],
                                  pxt[:, ts(r, BL)])

            h_in = hb[t % NH]
            h_out = hb[(t + 1) % NH]
            c_in = c_st[t % 2]
            c_out = c_st[(t + 1) % 2]

            # z_k = wcat[:, 64k:64k+64]^T @ [h; x; 1]   (bias via ones row)
            pzif = pz_pool.tile([HID, 2 * BL], f32, name="pzif", space="PSUM")
            pzgo = pz_pool.tile([HID, 2 * BL], f32, name="pzgo", space="PSUM")
            nc.tensor.matmul(pzif[:, 0:BL], lhsT=wcat_sb[:, 0:HID],
                             rhs=h_in[:], start=True, stop=True)
            nc.tensor.matmul(pzif[:, BL:2 * BL], lhsT=wcat_sb[:, HID:2 * HID],
                             rhs=h_in[:], start=True, stop=True)
            nc.tensor.matmul(pzgo[:, 0:BL], lhsT=wcat_sb[:, 2 * HID:3 * HID],
                             rhs=h_in[:], start=True, stop=True)
            nc.tensor.matmul(pzgo[:, BL:2 * BL], lhsT=wcat_sb[:, 3 * HID:4 * HID],
                             rhs=h_in[:], start=True, stop=True)

            # gates
            sif = g_pool.tile([HID, 2 * BL], f32, name="sif")
            nc.scalar.activation(sif[:], pzif[:],
                                 mybir.ActivationFunctionType.Sigmoid)
            tg = g_pool.tile([HID, BL], f32, name="tg")
            nc.scalar.activation(tg[:], pzgo[:, 0:BL],
                                 mybir.ActivationFunctionType.Tanh)
            so = g_pool.tile([HID, BL], f32, name="so")
            nc.scalar.activation(so[:], pzgo[:, BL:2 * BL],
                                 mybir.ActivationFunctionType.Sigmoid)

            # c' = f*c + i*g~ ; h' = o * tanh(c')
            v = tmp_pool.tile([HID, BL], f32, name="v")
            nc.vector.tensor_mul(v[:], sif[:, BL:2 * BL], c_in[:])
            u = tmp_pool.tile([HID, BL], f32, name="u")
            nc.vector.tensor_mul(u[:], sif[:, 0:BL], tg[:])
            nc.vector.tensor_add(c_out[:], u[:], v[:])
            thc = tmp_pool.tile([HID, BL], f32, name="thc")
            nc.scalar.activation(thc[:], c_out[:],
                                 mybir.ActivationFunctionType.Tanh)
            nc.vector.tensor_mul(h_out[0:HID, :], so[:], thc[:])

        # ---- dense head + softmax ----
        h_fin = hb[t_steps % NH]
        hf32 = head_pool.tile([KC, BL], f32, name="hf32")
        nc.vector.tensor_copy(hf32[:], h_fin[:])
        plog = phead_pool.tile([BL, NCLS], f32, name="plog", space="PSUM")
        nc.tensor.matmul(plog[:], lhsT=hf32[:], rhs=wdb_sb[:], start=True,
                         stop=True)
        e = head_pool.tile([BL, NCLS], f32, name="e")
        nc.scalar.activation(e[:], plog[:], mybir.ActivationFunctionType.Exp)
        s = head_pool.tile([BL, 1], f32, name="s")
        nc.vector.tensor_reduce(s[:], e[:], axis=mybir.AxisListType.X,
                                op=mybir.AluOpType.add)
        rcp = head_pool.tile([BL, 1], f32, name="rcp")
        nc.vector.reciprocal(rcp[:], s[:])
        prob = head_pool.tile([BL, NCLS], f32, name="prob")
        nc.vector.tensor_scalar(prob[:], e[:], rcp[:, 0:1], None,
                                mybir.AluOpType.mult)
        nc.sync.dma_start(out_p[:], prob[:])

    nc.compile()
    return nc


def _host_prep(inputs, t_steps=T):
    import ml_dtypes
    bf = ml_dtypes.bfloat16
    tokens = np.ascontiguousarray(np.asarray(inputs["tokens"]).astype(np.int32))
    emb = np.ascontiguousarray(
        np.asarray(inputs["emb"], dtype=np.float32).astype(bf))
    Wk = np.asarray(inputs["Wk"], dtype=np.float32)
    Wr = np.asarray(inputs["Wr"], dtype=np.float32)
    b = np.asarray(inputs["b"], dtype=np.float32)
    Wd = np.asarray(inputs["Wd"], dtype=np.float32)
    bd = np.asarray(inputs["bd"], dtype=np.float32)

    # rhs rows: 0:64 h -> Wr, 64:96 x -> Wk, 96 ones -> b / bd
    wcat = np.ascontiguousarray(
        np.concatenate([Wr, Wk, b[None, :]], axis=0).astype(np.float32)
        .astype(bf))
    wdb = np.ascontiguousarray(np.concatenate(
        [Wd, np.zeros((EMB, NCLS), np.float32), bd[None, :]],
        axis=0).astype(np.float32))

    in_maps = []
    for c in range(NCORES):
        shard = tokens[c * BL:(c + 1) * BL, :t_steps]  # [64, T]
        # tok2[r*64 + b, j] = shard[b, 2j + r]
        tok2 = np.ascontiguousarray(
            shard.reshape(BL, t_steps // 2, 2).transpose(2, 0, 1)
            .reshape(2 * BL, t_steps // 2))
        in_maps.append({"tok2": tok2, "emb": emb, "wcat": wcat, "wdb": wdb})
    return in_maps


def kernel(**inputs) -> np.ndarray:
    from concourse.bass_utils import run_bass_kernel_spmd

    if "prog" not in _CACHE:
        _CACHE["prog"] = build_program(T)
    nc = _CACHE["prog"]

    in_maps = _host_prep(inputs, T)
    res = run_bass_kernel_spmd(nc, in_maps, list(range(NCORES)))
    outs = [np.asarray(res.results[c]["out"]) for c in range(NCORES)]
    return np.concatenate(outs, axis=0).astype(np.float32)


# revision 10
# speedup vs baseline: 1.3059x; 1.0332x over previous
# Trainium2 Bass kernel for: embedding -> LSTM (last hidden) -> dense -> softmax
#
#   tokens [512, 512] int  -> emb lookup [B, T, 32] -> LSTM(64) last hidden
#   -> dense(3) -> softmax  => out [512, 3] f32
#
# Sharding: data-parallel over batch across 8 cores (64 rows each); embedding
# table + weights replicated.
#
# Per-core device program (fully unrolled over T=512 steps):
#   - gather emb rows for 2 timesteps at a time via indirect DMA -> [128, 32],
#     PE-transpose to x^T [32, 128], DMA the per-step [32, 64] slice into rows
#     64:96 of the step's rhs tile
#   - rhs tile [97, 64] = [h_t ; x_t^T ; 1]; one K=97 matmul per gate against
#     wcat = vstack(Wr, Wk, b) column slices -> z_i z_f in one psum tile,
#     z_g z_o in another (gates along the free dim so every elementwise op
#     stays on partitions 0:64)
#   - ACT: sigmoid(i|f) in one op, tanh(g), sigmoid(o); DVE: c' = f*c + i*g~,
#     h' = o * tanh(c') written straight into the next rhs tile
#   - dense head: one K=97 matmul with wdb = vstack(Wd, 0, bd) (the x rows
#     multiply zeros, the ones row adds bd), then softmax on device.

import numpy as np

VOCAB, EMB, HID, NCLS, B, T = 50000, 32, 64, 3, 512, 512
NCORES = 8
BL = B // NCORES  # 64 batch rows per core
KC = HID + EMB + 1  # 97: h rows, x rows, ones row
NH = 4  # h/rhs tile ring depth

_CACHE = {}


def build_program(t_steps=T):
    from contextlib import ExitStack

    import concourse.bass as bass
    import concourse.mybir as mybir
    import concourse.tile as tile
    from concourse import bacc
    from concourse.bass import ts
    from concourse.masks import make_identity

    f32 = mybir.dt.float32
    bf16 = mybir.dt.bfloat16
    i32 = mybir.dt.int32
    npairs = t_steps // 2

    nc = bacc.Bacc("TRN2", target_bir_lowering=False, debug=False,
                   num_devices=NCORES)

    tok2_p = nc.declare_dram_parameter("tok2", [2 * BL, npairs], i32,
                                       isOutput=False)
    emb_p = nc.declare_dram_parameter("emb", [VOCAB, EMB], bf16, isOutput=False)
    wcat_p = nc.declare_dram_parameter("wcat", [KC, 4 * HID], bf16,
                                       isOutput=False)
    wdb_p = nc.declare_dram_parameter("wdb", [KC, NCLS], f32, isOutput=False)
    out_p = nc.declare_dram_parameter("out", [BL, NCLS], f32, isOutput=True)

    with ExitStack() as ctx:
        tc = ctx.enter_context(tile.TileContext(nc))
        consts = ctx.enter_context(tc.tile_pool(name="consts", bufs=1))
        state = ctx.enter_context(tc.tile_pool(name="state", bufs=1))
        gath_pool = ctx.enter_context(tc.tile_pool(name="gath", bufs=8))
        g_pool = ctx.enter_context(tc.tile_pool(name="gates", bufs=3))
        tmp_pool = ctx.enter_context(tc.tile_pool(name="tmps", bufs=3))
        pz_pool = ctx.enter_context(tc.tile_pool(name="pz", bufs=2,
                                                 space="PSUM"))
        pxt_pool = ctx.enter_context(tc.tile_pool(name="pxt", bufs=3,
                                                  space="PSUM"))
        head_pool = ctx.enter_context(tc.tile_pool(name="head", bufs=1))
        phead_pool = ctx.enter_context(tc.tile_pool(name="phead", bufs=1,
                                                    space="PSUM"))

        # ---- constants / weights in SBUF ----
        tok_sb = consts.tile([2 * BL, npairs], i32, name="tok_sb")
        nc.sync.dma_start(tok_sb[:], tok2_p[:])
        wcat_sb = consts.tile([KC, 4 * HID], bf16, name="wcat_sb")
        nc.sync.dma_start(wcat_sb[:], wcat_p[:])
        wdb_sb = consts.tile([KC, NCLS], f32, name="wdb_sb")
        nc.sync.dma_start(wdb_sb[:], wdb_p[:])
        ident = consts.tile([128, 128], bf16, name="ident")
        make_identity(nc, ident[:])

        # ---- persistent state ----
        # rhs ring: [h ; x^T ; 1] tiles; c ping-pong.
        hb = [state.tile([KC, BL], bf16, name=f"hb{k}") for k in range(NH)]
        c_st = [state.tile([HID, BL], f32, name=f"c{k}") for k in (0, 1)]
        nc.vector.memset(hb[0][0:HID, :], 0.0)
        for k in range(NH):
            nc.vector.memset(hb[k][HID + EMB:KC, :], 1.0)
        nc.vector.memset(c_st[0][:], 0.0)

        pxt = None
        for t in range(t_steps):
            j, r = divmod(t, 2)
            if r == 0:
                # gather emb rows for steps (2j, 2j+1): row p of gath is
                # emb[tokens[p % 64, 2j + p // 64]]
                gath = gath_pool.tile([2 * BL, EMB], bf16, name="gath")
                nc.gpsimd.indirect_dma_start(
                    out=gath[:],
                    out_offset=None,
                    in_=emb_p[:],
                    in_offset=bass.IndirectOffsetOnAxis(
                        ap=tok_sb[:, j:j + 1], axis=0),
                )
                # transpose -> [EMB, 128]: cols 0:64 = x_{2j}^T, rest x_{2j+1}^T
                pxt = pxt_pool.tile([EMB, 2 * BL], bf16, name="pxt",
                                    space="PSUM")
                nc.tensor.matmul(pxt[:], lhsT=gath[:], rhs=ident[:],
                                 is_transpose=True, start=True, stop=True)
            # x_t^T into rows 64:96 of this step's rhs tile (partition-shifted
            # copy)
            nc.vector.tensor_copy(hb[t % NH][HID:HID + EMB, :],
                                  pxt[:, ts(r, BL)])

            h_in = hb[t % NH]
            h_out = hb[(t + 1) % NH]
            c_in = c_st[t % 2]
            c_out = c_st[(t + 1) % 2]

            # z_k = wcat[:, 64k:64k+64]^T @ [h; x; 1]   (bias via ones row)
            pzif = pz_pool.tile([HID, 2 * BL], f32, name="pzif", space="PSUM")
            pzgo = pz_pool.tile([HID, 2 * BL], f32, name="pzgo", space="PSUM")
            nc.tensor.matmul(pzif[:, 0:BL], lhsT=wcat_sb[:, 0:HID],
                             rhs=h_in[:], start=True, stop=True)
            nc.tensor.matmul(pzif[:, BL:2 * BL], lhsT=wcat_sb[:, HID:2 * HID],
                             rhs=h_in[:], start=True, stop=True)
            nc.tensor.matmul(pzgo[:, 0:BL], lhsT=wcat_sb[:, 2 * HID:3 * HID],
                             rhs=h_in[:], start=True, stop=True)
            nc.tensor.matmul(pzgo[:, BL:2 * BL], lhsT=wcat_sb[:, 3 * HID:4 * HID],
                             rhs=h_in[:], start=True, stop=True)

            # gates
            sif = g_pool.tile([HID, 2 * BL], bf16, name="sif")
            nc.scalar.activation(sif[:], pzif[:],
                                 mybir.ActivationFunctionType.Sigmoid)
            tg = g_pool.tile([HID, BL], bf16, name="tg")
            nc.scalar.activation(tg[:], pzgo[:, 0:BL],
                                 mybir.ActivationFunctionType.Tanh)
            so = g_pool.tile([HID, BL], bf16, name="so")
            nc.scalar.activation(so[:], pzgo[:, BL:2 * BL],
                                 mybir.ActivationFunctionType.Sigmoid)

            # c' = f*c + i*g~ ; h' = o * tanh(c')
            v = tmp_pool.tile([HID, BL], bf16, name="v")
            nc.vector.tensor_mul(v[:], sif[:, BL:2 * BL], c_in[:])
            u = tmp_pool.tile([HID, BL], bf16, name="u")
            nc.vector.tensor_mul(u[:], sif[:, 0:BL], tg[:])
            nc.vector.tensor_add(c_out[:], u[:], v[:])
            thc = tmp_pool.tile([HID, BL], bf16, name="thc")
            nc.scalar.activation(thc[:], c_out[:],
                                 mybir.ActivationFunctionType.Tanh)
            nc.vector.tensor_mul(h_out[0:HID, :], so[:], thc[:])

        # ---- dense head + softmax ----
        h_fin = hb[t_steps % NH]
        hf32 = head_pool.tile([KC, BL], f32, name="hf32")
        nc.vector.tensor_copy(hf32[:], h_fin[:])
        plog = phead_pool.tile([BL, NCLS], f32, name="plog", space="PSUM")
        nc.tensor.matmul(plog[:], lhsT=hf32[:], rhs=wdb_sb[:], start=True,
                         stop=True)
        e = head_pool.tile([BL, NCLS], f32, name="e")
        nc.scalar.activation(e[:], plog[:], mybir.ActivationFunctionType.Exp)
        s = head_pool.tile([BL, 1], f32, name="s")
        nc.vector.tensor_reduce(s[:], e[:], axis=mybir.AxisListType.X,
                                op=mybir.AluOpType.add)
        rcp = head_pool.tile([BL, 1], f32, name="rcp")
        nc.vector.reciprocal(rcp[:], s[:])
        prob = head_pool.tile([BL, NCLS], f32, name="prob")
        nc.vector.tensor_scalar(prob[:], e[:], rcp[:, 0:1], None,
                                mybir.AluOpType.mult)
        nc.sync.dma_start(out_p[:], prob[:])

    nc.compile()
    return nc


def _host_prep(inputs, t_steps=T):
    import ml_dtypes
    bf = ml_dtypes.bfloat16
    tokens = np.ascontiguousarray(np.asarray(inputs["tokens"]).astype(np.int32))
    emb = np.ascontiguousarray(
        np.asarray(inputs["emb"], dtype=np.float32).astype(bf))
    Wk = np.asarray(inputs["Wk"], dtype=np.float32)
    Wr = np.asarray(inputs["Wr"], dtype=np.float32)
    b = np.asarray(inputs["b"], dtype=np.float32)
    Wd = np.asarray(inputs["Wd"], dtype=np.float32)
    bd = np.asarray(inputs["bd"], dtype=np.float32)

    # rhs rows: 0:64 h -> Wr, 64:96 x -> Wk, 96 ones -> b / bd
    wcat = np.ascontiguousarray(
        np.concatenate([Wr, Wk, b[None, :]], axis=0).astype(np.float32)
        .astype(bf))
    wdb = np.ascontiguousarray(np.concatenate(
        [Wd, np.zeros((EMB, NCLS), np.float32), bd[None, :]],
        axis=0).astype(np.float32))

    in_maps = []
    for c in range(NCORES):
        shard = tokens[c * BL:(c + 1) * BL, :t_steps]  # [64, T]
        # tok2[r*64 + b, j] = shard[b, 2j + r]
        tok2 = np.ascontiguousarray(
            shard.reshape(BL, t_steps // 2, 2).transpose(2, 0, 1)
            .reshape(2 * BL, t_steps // 2))
        in_maps.append({"tok2": tok2, "emb": emb, "wcat": wcat, "wdb": wdb})
    return in_maps


def kernel(**inputs) -> np.ndarray:
    from concourse.bass_utils import run_bass_kernel_spmd

    if "prog" not in _CACHE:
        _CACHE["prog"] = build_program(T)
    nc = _CACHE["prog"]

    in_maps = _host_prep(inputs, T)
    res = run_bass_kernel_spmd(nc, in_maps, list(range(NCORES)))
    outs = [np.asarray(res.results[c]["out"]) for c in range(NCORES)]
    return np.concatenate(outs, axis=0).astype(np.float32)
